# revision 12
# baseline (speedup 1.0000x reference)
"""Causal self-attention (muP scaling) for Trainium2, sharded over 8 NeuronCores.

Sharding: data-parallel over batch (B=2) x tensor-parallel over head groups
(16 heads -> 4 groups of 4). Core c handles batch c//4, head group c%4.
Each core computes q/k/v projections for its 256 features, causal attention
for its 4 heads, and a row-parallel partial of the output projection
(written back in bf16; the host sums the 4 partials per batch element).

Dtype strategy (validated against the 2e-2 tolerance; measured 5.8e-3):
 - q,k projections and S=K^T Q run in fp8e4 with the DoubleRow perf mode
   (two 128-deep k-tiles contracted per pass). Weights are pre-scaled by
   32 on the host so their values sit in fp8's sweet spot; the 32*32
   factor is folded into the exp scale. S contracts the 64 features of a
   head as two 32-partition k-tiles, using PE tiling at base partition
   32h, which requires the q/k features laid out [32h + f%32, f//32] --
   the host permutes wq/wk columns accordingly.
 - v projection runs in fp8 DoubleRow with residual-stacked operand pairs
   (x8*wv8 + xr8*wv8 + x8*wvr8), giving bf16-grade accuracy at 0.75x the
   bf16 cost. The 32x weight scale cancels out through the softmax
   normalisation because the ones-column in V' is set to 32.0.
 - exp output (P), V', y and wproj are bf16; PV and the output projection
   are bf16 matmuls. PSUM accumulation is f32 throughout.

Pipeline per 256-query tile j (Tq=256 so S PSUM groups are 2 banks and can
be double-buffered): q/k/v chains -> per key-chunk i: 4 S matmuls into a
[128,4,256] PSUM group, one batched exp over all 4 heads straight out of
PSUM (bf16 out), causal masking only on the two diagonal chunks, 4 PV
accumulations into a [65,4,256] PSUM group whose ones-row yields the
softmax denominator for free. Deferred output-projection chunks of tile
j-1 are interleaved between key-chunks to fill TensorE gaps while ACT
grinds exp.
"""

import os
import sys

for _p in ("/opt/trn_rl_repo",):
    if _p not in sys.path:
        sys.path.insert(0, _p)

import numpy as np
import ml_dtypes

import concourse.bass as bass  # noqa: F401
import concourse.mybir as mybir
import concourse.tile as tile
from concourse import bacc
from concourse.bass_utils import run_bass_kernel_spmd
from concourse.masks import make_upper_triangular
from concourse.tile import ScopedClock

# ---- problem constants (hardcoded per contract) ----
B, T, C = 2, 2048, 1024
NH, DH = 16, 64
N_CORES = 8
GROUPS = 4                 # head groups (tensor parallel)
NH_LOC = NH // GROUPS      # 4 heads per core
F = NH_LOC * DH            # 256 per-core qkv features
P = 128
TQ = 256                   # query tile
NJ = T // TQ               # 8 query tiles
NTC = T // P               # 16 key chunks of 128
NKK = C // 256             # 4 DoubleRow k-tile pairs over C
f32 = mybir.dt.float32
bf16 = mybir.dt.bfloat16
f8 = mybir.dt.float8e4
EXP = mybir.ActivationFunctionType.Exp
DR = mybir.MatmulPerfMode.DoubleRow
F8 = ml_dtypes.float8_e4m3
BF = ml_dtypes.bfloat16


def _install_drain_patch():
    """This walrus build rejects >2 sem waits on a single instruction; the
    Tile tail drain accumulates one wait per live proc. Split them into
    single-wait SP nops ahead of the drain."""
    if getattr(tile.TileContext, "_drain_patch_installed", False):
        return

    def _patched(self, tick_clock, wait_clock):
        nc = self.nc
        probe = nc.sync.nop(nofuse=True)
        wait_clock.add_sem_waits(
            probe.ins, ScopedClock({None: tick_clock.global_clock})
        )
        si = probe.ins.sync_info
        waits = list(si.on_wait) if si is not None and si.on_wait else []
        if len(waits) > 1:
            probe.ins.sync_info.on_wait = [waits[0]]
            for w in waits[1:]:
                n2 = nc.sync.nop(nofuse=True)
                n2.ins.sync_info = mybir.SyncInfo(on_wait=[w], on_update=[])
        nc.sync.drain()
        nc.all_engine_barrier()
        assert self.sems is not None
        popped = nc._tile_sem_poison_stack.pop()
        assert popped is self._sem_poison
        nc.clear_and_free_semaphores(list(self.sems.allocated().values()))
        nc.all_engine_barrier()

    tile.TileContext._drain_and_barrier = _patched
    tile.TileContext._drain_patch_installed = True


def build_module():
    """Build the per-core Bass module (uniform across all 8 cores)."""
    _install_drain_patch()
    nc = bacc.Bacc("TRN2", target_bir_lowering=False, debug=False)
    x8 = nc.dram_tensor("x8", [C, T], f8, kind="ExternalInput").ap()
    xr8 = nc.dram_tensor("xr8", [C, T], f8, kind="ExternalInput").ap()
    wq8 = nc.dram_tensor("wq8", [C, F], f8, kind="ExternalInput").ap()
    wk8 = nc.dram_tensor("wk8", [C, F], f8, kind="ExternalInput").ap()
    wv8 = nc.dram_tensor("wv8", [C, F], f8, kind="ExternalInput").ap()
    wvr8 = nc.dram_tensor("wvr8", [C, F], f8, kind="ExternalInput").ap()
    wpt = nc.dram_tensor("wpt", [F, C], bf16, kind="ExternalInput").ap()
    out = nc.dram_tensor("out", [T, C], bf16, kind="ExternalOutput").ap()

    with tile.TileContext(nc) as tc:
        _body(tc, x8, xr8, wq8, wk8, wv8, wvr8, wpt, out)
    nc.compile()
    return nc


def _body(tc, x8, xr8, wq8, wk8, wv8, wvr8, wpt, out):
    from contextlib import ExitStack

    nc = tc.nc
    with ExitStack() as ctx:
        const = ctx.enter_context(tc.tile_pool(name="const", bufs=1))
        wpool = ctx.enter_context(tc.tile_pool(name="wpool", bufs=1))
        qkv = ctx.enter_context(tc.tile_pool(name="qkv", bufs=1))
        sexp = ctx.enter_context(tc.tile_pool(name="sexp", bufs=6))
        ytp = ctx.enter_context(tc.tile_pool(name="ytp", bufs=2))
        rbp = ctx.enter_context(tc.tile_pool(name="rbp", bufs=2))
        small = ctx.enter_context(tc.tile_pool(name="small", bufs=2))
        outp = ctx.enter_context(tc.tile_pool(name="outp", bufs=4))
        ps_s = ctx.enter_context(tc.tile_pool(name="ps_s", bufs=2, space="PSUM"))
        ps_y = ctx.enter_context(tc.tile_pool(name="ps_y", bufs=1, space="PSUM"))
        ps_1 = ctx.enter_context(tc.tile_pool(name="ps_1", bufs=1, space="PSUM"))
        ps_3 = ctx.enter_context(tc.tile_pool(name="ps_3", bufs=1, space="PSUM"))

        # causal mask for the diagonal 128-blocks, replicated over 4 heads
        umask = const.tile([P, P], bf16)
        make_upper_triangular(nc, umask, val=1.0, diag=True)
        m4 = const.tile([P, NH_LOC, P], bf16)
        for h in range(NH_LOC):
            nc.gpsimd.tensor_copy(m4[:, h, :], umask)

        # ---- weights + x straight from HBM (pre-transposed, fp8/bf16) ----
        wq8t = wpool.tile([P, NKK, 2, F], f8)
        wk8t = wpool.tile([P, NKK, 2, F], f8)
        wv8t = wpool.tile([P, NKK, 2, F], f8)
        wvr8t = wpool.tile([P, NKK, 2, F], f8)
        wptt = wpool.tile([P, 2, C], bf16)
        x8t = wpool.tile([P, NKK, 2, T], f8)
        xr8t = wpool.tile([P, NKK, 2, T], f8)

        wq_r = wq8.rearrange("(kk two p) m -> p kk two m", p=P, two=2)
        wk_r = wk8.rearrange("(kk two p) m -> p kk two m", p=P, two=2)
        wv_r = wv8.rearrange("(kk two p) m -> p kk two m", p=P, two=2)
        wvr_r = wvr8.rearrange("(kk two p) m -> p kk two m", p=P, two=2)
        x8_r = x8.rearrange("(kk two p) t -> p kk two t", p=P, two=2)
        xr8_r = xr8.rearrange("(kk two p) t -> p kk two t", p=P, two=2)

        nc.sync.dma_start(out=wq8t, in_=wq_r)
        nc.sync.dma_start(out=wk8t, in_=wk_r)
        sl = slice(0, 256)
        nc.sync.dma_start(out=x8t[:, :, :, sl], in_=x8_r[:, :, :, sl])
        nc.sync.dma_start(out=wv8t, in_=wv_r)
        nc.sync.dma_start(out=wvr8t, in_=wvr_r)
        nc.sync.dma_start(out=xr8t[:, :, :, sl], in_=xr8_r[:, :, :, sl])
        sl = slice(256, 512)
        nc.sync.dma_start(out=x8t[:, :, :, sl], in_=x8_r[:, :, :, sl])
        nc.sync.dma_start(out=xr8t[:, :, :, sl], in_=xr8_r[:, :, :, sl])
        for tch in range(1, 4):
            sl = slice(tch * 512, (tch + 1) * 512)
            nc.sync.dma_start(out=x8t[:, :, :, sl], in_=x8_r[:, :, :, sl])
            nc.sync.dma_start(out=xr8t[:, :, :, sl], in_=xr8_r[:, :, :, sl])
        nc.sync.dma_start(out=wptt, in_=wpt.rearrange("(fc p) o -> p fc o", p=P))

        # ---- persistent q^T, k^T (fp8, 32-feature split) and V' (bf16) ----
        qT8 = qkv.tile([P, 2, T], f8)
        kT8 = qkv.tile([P, 2, T], f8)
        Vp = qkv.tile([P, NTC, NH_LOC * (DH + 1)], bf16)
        # ones-column = 32.0: cancels the 32x weight scale on V through the
        # softmax denominator (py row 64 = 32*Z, y rows = 32*y_raw)
        for h in range(NH_LOC):
            nc.gpsimd.memset(Vp[:, :, h * (DH + 1) + DH], 32.0)

        scale = 1.0 / float(DH * 1024)  # muP 1/dh plus the 32*32 fp8 scale
        ps1t = ps_1.tile([P, 2, TQ], f32)
        ps3t = ps_3.tile([P, 2, TQ], f32)

        s1_slot = [0]

        def q_chain(j, half, dst):
            s1_slot[0] ^= 1
            pq = ps1t[:, s1_slot[0], :]
            for kk in range(NKK):
                nc.tensor.matmul(
                    pq,
                    lhsT=wq8t[:, kk, :, half * P:(half + 1) * P] if dst is qT8
                    else wk8t[:, kk, :, half * P:(half + 1) * P],
                    rhs=x8t[:, kk, :, j * TQ:(j + 1) * TQ],
                    start=(kk == 0),
                    stop=(kk == NKK - 1),
                    perf_mode=DR,
                )
            nc.vector.tensor_copy(dst[:, half, j * TQ:(j + 1) * TQ], pq)

        def v_chain(j, r):
            # natural-layout v for key chunk 2j+r via 3-term fp8 residual
            s1_slot[0] ^= 1
            pv = ps1t[:, s1_slot[0], :]
            tsl = slice((2 * j + r) * P, (2 * j + r + 1) * P)
            for term, (xt, wt) in enumerate(
                ((x8t, wv8t), (xr8t, wv8t), (x8t, wvr8t))
            ):
                for kk in range(NKK):
                    nc.tensor.matmul(
                        pv[:, 0:F],
                        lhsT=xt[:, kk, :, tsl],
                        rhs=wt[:, kk, :, :],
                        start=(term == 0 and kk == 0),
                        stop=(term == 2 and kk == NKK - 1),
                        perf_mode=DR,
                    )
            nc.vector.tensor_copy(
                Vp[:, 2 * j + r].rearrange("p (h c) -> p h c", c=DH + 1)[
                    :, :, 0:DH
                ],
                pv.rearrange("p (h c) -> p h c", c=DH)[:, 0:NH_LOC, :],
            )

        pending = []

        def proj_q(j, yts, q):
            # row-parallel output projection: one 128-query strip, four
            # [128,256] chunks through the two ps3t slots, paired [128,512]
            # evacuations and early DMAs
            for pair in range(2):
                ob = outp.tile([P, 512], bf16, tag="ob", name=f"ob_{j}_{q}_{pair}")
                for s in range(2):
                    n = 2 * pair + s
                    po = ps3t[:, s, :]
                    for fc in range(2):
                        nc.tensor.matmul(
                            po,
                            lhsT=yts[fc][:, q * P:(q + 1) * P],
                            rhs=wptt[:, fc, n * 256:(n + 1) * 256],
                            start=(fc == 0),
                            stop=(fc == 1),
                        )
                nc.vector.tensor_copy(
                    ob, ps3t.rearrange("p two t -> p (two t)")
                )
                nc.sync.dma_start(
                    out=out[
                        j * TQ + q * P: j * TQ + (q + 1) * P,
                        pair * 512:(pair + 1) * 512,
                    ],
                    in_=ob,
                )

        # prologue: tile 0 needs its projections before its two groups
        q_chain(0, 0, qT8)
        q_chain(0, 1, qT8)
        q_chain(0, 0, kT8)
        q_chain(0, 1, kT8)
        v_chain(0, 0)
        v_chain(0, 1)

        # ---- stage 2: one flat software pipeline over all (j, i) groups.
        # PV(u) is emitted 3 units after S(u) so the in-order PE stream keeps
        # feeding ACT across tile boundaries; v chains, next-tile q/k chains
        # and deferred projection strips fill the remaining TensorE gaps.
        tiles = {}          # j -> (py, yts)

        def y_norm(j):
            # yts = py[0:64] * (1/(32 Z)) partition-broadcast
            py, yts = tiles.pop(j)
            rc = small.tile([1, NH_LOC, TQ], f32, tag="rc")
            nc.vector.reciprocal(rc, py[DH:DH + 1, :, :])
            rb = rbp.tile([DH, NH_LOC, TQ], f32, tag="rb")
            nc.gpsimd.partition_broadcast(rb, rc)
            for h in range(NH_LOC):
                fc, ro = h // 2, (h % 2) * DH
                nc.vector.tensor_mul(
                    yts[fc][ro:ro + DH, :], py[0:DH, h, :], rb[:, h, :]
                )
            for q in range(TQ // P):
                fillers.append((NJ, lambda j=j, yts=yts, q=q: proj_q(j, yts, q)))

        def pv_group(j, i, se, start):
            # even heads open their bank with start=True, which marks the
            # whole bank pending-zero; the odd head's first write then
            # auto-overwrites (acts as its own start). Group bookkeeping
            # is skipped -- the two interleaved chains share a bank.
            py = tiles[j][0]
            for h in range(NH_LOC):
                nc.tensor.matmul(
                    py[:, h, start:],
                    lhsT=Vp[:, i, h * (DH + 1):(h + 1) * (DH + 1)],
                    rhs=se[:, h, start:],
                    start=(i == 0 and h % 2 == 0),
                    stop=False,
                    skip_group_check=True,
                )
            if i == 2 * j + 1:
                y_norm(j)

        fillers = []        # (deadline_tile, closure), popped one per unit
        pv_q = []
        units = [(j, i) for j in range(NJ) for i in range(2 * j + 2)]
        for (j, i) in units:
            if i == 0:
                tiles[j] = (
                    ps_y.tile([DH + 1, NH_LOC, TQ], f32, tag="y",
                              name=f"py_{j}"),
                    [ytp.tile([P, TQ], bf16, tag=f"yt{fc}", name=f"yt{fc}_{j}")
                     for fc in range(2)],
                )
                if j > 0:
                    # V chunks 2j, 2j+1 are read by this tile's last two
                    # groups; the q/k chains feed tile j+1's first group
                    fillers.append((j, lambda j=j: v_chain(j, 0)))
                    fillers.append((j, lambda j=j: v_chain(j, 1)))
                if j + 1 < NJ:
                    for half, dst in ((0, qT8), (1, qT8), (0, kT8), (1, kT8)):
                        fillers.append(
                            (j + 1,
                             lambda j=j, half=half, dst=dst:
                                 q_chain(j + 1, half, dst))
                        )
                # flush anything that must precede this tile's groups
                overdue = [f for (dl, f) in fillers if dl <= j]
                fillers = [(dl, f) for (dl, f) in fillers if dl > j]
                for f in overdue:
                    f()

            d = i - 2 * j
            start = 0 if d <= 0 else 128
            ps = ps_s.tile([P, NH_LOC, TQ], f32, tag="s")
            for h in range(NH_LOC):
                nc.tensor.matmul(
                    ps[:, h, start:],
                    lhsT=kT8[32 * h:32 * h + 32, :, i * P:(i + 1) * P],
                    rhs=qT8[32 * h:32 * h + 32, :, j * TQ + start:(j + 1) * TQ],
                    start=True,
                    stop=True,
                    perf_mode=DR,
                    tile_position=(32 * h, 0),
                )
            se = sexp.tile([P, NH_LOC, TQ], bf16, tag="se")
            nc.scalar.activation(se[:, :, start:], ps[:, :, start:], EXP,
                                 scale=scale)
            if d == 0:
                nc.gpsimd.tensor_mul(se[:, :, 0:P], se[:, :, 0:P], m4)
            elif d == 1:
                nc.gpsimd.tensor_mul(se[:, :, P:TQ], se[:, :, P:TQ], m4)
            pv_q.append((j, i, se, start))
            if len(pv_q) > 3:
                pv_group(*pv_q.pop(0))
            if fillers:
                fillers.pop(0)[1]()

        for args in pv_q:
            pv_group(*args)
        for _, fill in fillers:
            fill()


_CACHE = {}


def _perm():
    # feature permutation for the 32-feature-split q/k layout:
    # column (half*128 + 32h + f) holds feature (64h + 32*half + f)
    perm = np.zeros(F, dtype=np.int64)
    for half in range(2):
        for h in range(NH_LOC):
            for f in range(32):
                perm[half * 128 + 32 * h + f] = 64 * h + 32 * half + f
    return perm


def shard_inputs(x, wq, wk, wv, wproj):
    perm = _perm()
    in_maps = []
    for c in range(N_CORES):
        b, g = divmod(c, GROUPS)
        sl = slice(g * F, (g + 1) * F)
        xb = np.ascontiguousarray(x[b].T, dtype=np.float32)  # [C, T]
        x8 = xb.astype(F8)
        xr8 = (xb - x8.astype(np.float32)).astype(F8)
        wqt = 32.0 * wq[sl, :].T  # [C, F]
        wkt = 32.0 * wk[sl, :].T
        wvt = 32.0 * wv[sl, :].T
        wv8 = wvt.astype(F8)
        wvr8 = (wvt - wv8.astype(np.float32)).astype(F8)
        in_maps.append(
            {
                "x8": x8,
                "xr8": xr8,
                "wq8": np.ascontiguousarray(wqt[:, perm]).astype(F8),
                "wk8": np.ascontiguousarray(wkt[:, perm]).astype(F8),
                "wv8": wv8,
                "wvr8": wvr8,
                "wpt": np.ascontiguousarray(wproj[:, sl].T).astype(BF),
            }
        )
    return in_maps


def kernel(x, wq, wk, wv, wproj):
    x = np.asarray(x, dtype=np.float32)
    wq = np.asarray(wq, dtype=np.float32)
    wk = np.asarray(wk, dtype=np.float32)
    wv = np.asarray(wv, dtype=np.float32)
    wproj = np.asarray(wproj, dtype=np.float32)

    from concourse._compat import axon_active

    if axon_active():
        # the axon NTFF-profile hook isn't available in this environment;
        # a BASS_TRACE=1 run would crash importing it, so disable tracing
        os.environ.setdefault("BASS_NEVER_TRACE", "1")

    if "nc" not in _CACHE:
        _CACHE["nc"] = build_module()
    nc = _CACHE["nc"]

    in_maps = shard_inputs(x, wq, wk, wv, wproj)
    res = run_bass_kernel_spmd(nc, in_maps, core_ids=list(range(N_CORES)))
    out = np.zeros((B, T, C), np.float32)
    for c in range(N_CORES):
        b = c // GROUPS
        out[b] += res.results[c]["out"].astype(np.float32)
    return out


# revision 14
# speedup vs baseline: 1.0424x; 1.0424x over previous
"""Causal self-attention (muP scaling) for Trainium2, sharded over 8 NeuronCores.

Sharding: data-parallel over batch (B=2) x tensor-parallel over head groups
(16 heads -> 4 groups of 4). Core c handles batch c//4, head group c%4.
Each core computes q/k/v projections for its 256 features, causal attention
for its 4 heads, and a row-parallel partial of the output projection
(written back in bf16; the host sums the 4 partials per batch element).

Dtype strategy (validated against the 2e-2 tolerance; measured 5.8e-3):
 - q,k projections and S=K^T Q run in fp8e4 with the DoubleRow perf mode
   (two 128-deep k-tiles contracted per pass). Weights are pre-scaled by
   32 on the host so their values sit in fp8's sweet spot; the 32*32
   factor is folded into the exp scale. S contracts the 64 features of a
   head as two 32-partition k-tiles, using PE tiling at base partition
   32h, which requires the q/k features laid out [32h + f%32, f//32] --
   the host permutes wq/wk columns accordingly.
 - v projection runs in fp8 DoubleRow with residual-stacked operand pairs
   (x8*wv8 + xr8*wv8 + x8*wvr8), giving bf16-grade accuracy at 0.75x the
   bf16 cost. The 32x weight scale cancels out through the softmax
   normalisation because the ones-column in V' is set to 32.0.
 - exp output (P), V', y and wproj are bf16; PV and the output projection
   are bf16 matmuls. PSUM accumulation is f32 throughout.

Pipeline per 256-query tile j (Tq=256 so S PSUM groups are 2 banks and can
be double-buffered): q/k/v chains -> per key-chunk i: 4 S matmuls into a
[128,4,256] PSUM group, one batched exp over all 4 heads straight out of
PSUM (bf16 out), causal masking only on the two diagonal chunks, 4 PV
accumulations into a [65,4,256] PSUM group whose ones-row yields the
softmax denominator for free. Deferred output-projection chunks of tile
j-1 are interleaved between key-chunks to fill TensorE gaps while ACT
grinds exp.
"""

import os
import sys

for _p in ("/opt/trn_rl_repo",):
    if _p not in sys.path:
        sys.path.insert(0, _p)

import numpy as np
import ml_dtypes

import concourse.bass as bass  # noqa: F401
import concourse.mybir as mybir
import concourse.tile as tile
from concourse import bacc
from concourse.bass_utils import run_bass_kernel_spmd
from concourse.masks import make_upper_triangular
from concourse.tile import ScopedClock

# ---- problem constants (hardcoded per contract) ----
B, T, C = 2, 2048, 1024
NH, DH = 16, 64
N_CORES = 8
GROUPS = 4                 # head groups (tensor parallel)
NH_LOC = NH // GROUPS      # 4 heads per core
F = NH_LOC * DH            # 256 per-core qkv features
P = 128
TQ = 256                   # query tile
NJ = T // TQ               # 8 query tiles
NTC = T // P               # 16 key chunks of 128
NKK = C // 256             # 4 DoubleRow k-tile pairs over C
f32 = mybir.dt.float32
bf16 = mybir.dt.bfloat16
f8 = mybir.dt.float8e4
EXP = mybir.ActivationFunctionType.Exp
DR = mybir.MatmulPerfMode.DoubleRow
F8 = ml_dtypes.float8_e4m3
BF = ml_dtypes.bfloat16


def _install_drain_patch():
    """This walrus build rejects >2 sem waits on a single instruction; the
    Tile tail drain accumulates one wait per live proc. Split them into
    single-wait SP nops ahead of the drain."""
    if getattr(tile.TileContext, "_drain_patch_installed", False):
        return

    def _patched(self, tick_clock, wait_clock):
        nc = self.nc
        probe = nc.sync.nop(nofuse=True)
        wait_clock.add_sem_waits(
            probe.ins, ScopedClock({None: tick_clock.global_clock})
        )
        si = probe.ins.sync_info
        waits = list(si.on_wait) if si is not None and si.on_wait else []
        if len(waits) > 1:
            probe.ins.sync_info.on_wait = [waits[0]]
            for w in waits[1:]:
                n2 = nc.sync.nop(nofuse=True)
                n2.ins.sync_info = mybir.SyncInfo(on_wait=[w], on_update=[])
        nc.sync.drain()
        nc.all_engine_barrier()
        assert self.sems is not None
        popped = nc._tile_sem_poison_stack.pop()
        assert popped is self._sem_poison
        nc.clear_and_free_semaphores(list(self.sems.allocated().values()))
        nc.all_engine_barrier()

    tile.TileContext._drain_and_barrier = _patched
    tile.TileContext._drain_patch_installed = True


def build_module():
    """Build the per-core Bass module (uniform across all 8 cores)."""
    _install_drain_patch()
    nc = bacc.Bacc("TRN2", target_bir_lowering=False, debug=False)
    x8 = nc.dram_tensor("x8", [C, T], f8, kind="ExternalInput").ap()
    xr8 = nc.dram_tensor("xr8", [C, T], f8, kind="ExternalInput").ap()
    wq8 = nc.dram_tensor("wq8", [C, F], f8, kind="ExternalInput").ap()
    wk8 = nc.dram_tensor("wk8", [C, F], f8, kind="ExternalInput").ap()
    wv8 = nc.dram_tensor("wv8", [C, F], f8, kind="ExternalInput").ap()
    wvr8 = nc.dram_tensor("wvr8", [C, F], f8, kind="ExternalInput").ap()
    wpt = nc.dram_tensor("wpt", [F, C], bf16, kind="ExternalInput").ap()
    out = nc.dram_tensor("out", [T, C], bf16, kind="ExternalOutput").ap()

    with tile.TileContext(nc) as tc:
        _body(tc, x8, xr8, wq8, wk8, wv8, wvr8, wpt, out)
    nc.compile()
    return nc


def _body(tc, x8, xr8, wq8, wk8, wv8, wvr8, wpt, out):
    from contextlib import ExitStack

    nc = tc.nc
    with ExitStack() as ctx:
        const = ctx.enter_context(tc.tile_pool(name="const", bufs=1))
        wpool = ctx.enter_context(tc.tile_pool(name="wpool", bufs=1))
        qkv = ctx.enter_context(tc.tile_pool(name="qkv", bufs=1))
        sexp = ctx.enter_context(tc.tile_pool(name="sexp", bufs=8))
        ytp = ctx.enter_context(tc.tile_pool(name="ytp", bufs=4))
        rbp = ctx.enter_context(tc.tile_pool(name="rbp", bufs=3))
        small = ctx.enter_context(tc.tile_pool(name="small", bufs=3))
        outp = ctx.enter_context(tc.tile_pool(name="outp", bufs=6))
        ps_s = ctx.enter_context(tc.tile_pool(name="ps_s", bufs=2, space="PSUM"))
        ps_y = ctx.enter_context(tc.tile_pool(name="ps_y", bufs=1, space="PSUM"))
        ps_1 = ctx.enter_context(tc.tile_pool(name="ps_1", bufs=1, space="PSUM"))
        ps_3 = ctx.enter_context(tc.tile_pool(name="ps_3", bufs=1, space="PSUM"))

        # causal mask for the diagonal 128-blocks, replicated over 4 heads
        umask = const.tile([P, P], f8)
        make_upper_triangular(nc, umask, val=1.0, diag=True)
        m4 = const.tile([P, NH_LOC, P], f8)
        for h in range(NH_LOC):
            nc.gpsimd.tensor_copy(m4[:, h, :], umask)

        # ---- weights + x straight from HBM (pre-transposed, fp8/bf16) ----
        wq8t = wpool.tile([P, NKK, 2, F], f8)
        wk8t = wpool.tile([P, NKK, 2, F], f8)
        wv8t = wpool.tile([P, NKK, 2, F], f8)
        wvr8t = wpool.tile([P, NKK, 2, F], f8)
        wptt = wpool.tile([P, 2, C], bf16)
        x8t = wpool.tile([P, NKK, 2, T], f8)
        xr8t = wpool.tile([P, NKK, 2, T], f8)

        wq_r = wq8.rearrange("(kk two p) m -> p kk two m", p=P, two=2)
        wk_r = wk8.rearrange("(kk two p) m -> p kk two m", p=P, two=2)
        wv_r = wv8.rearrange("(kk two p) m -> p kk two m", p=P, two=2)
        wvr_r = wvr8.rearrange("(kk two p) m -> p kk two m", p=P, two=2)
        x8_r = x8.rearrange("(kk two p) t -> p kk two t", p=P, two=2)
        xr8_r = xr8.rearrange("(kk two p) t -> p kk two t", p=P, two=2)

        nc.sync.dma_start(out=wq8t, in_=wq_r)
        nc.sync.dma_start(out=wk8t, in_=wk_r)
        sl = slice(0, 256)
        nc.sync.dma_start(out=x8t[:, :, :, sl], in_=x8_r[:, :, :, sl])
        nc.sync.dma_start(out=wv8t, in_=wv_r)
        nc.sync.dma_start(out=wvr8t, in_=wvr_r)
        nc.sync.dma_start(out=xr8t[:, :, :, sl], in_=xr8_r[:, :, :, sl])
        sl = slice(256, 512)
        nc.sync.dma_start(out=x8t[:, :, :, sl], in_=x8_r[:, :, :, sl])
        nc.sync.dma_start(out=xr8t[:, :, :, sl], in_=xr8_r[:, :, :, sl])
        for tch in range(1, 4):
            sl = slice(tch * 512, (tch + 1) * 512)
            nc.sync.dma_start(out=x8t[:, :, :, sl], in_=x8_r[:, :, :, sl])
            nc.sync.dma_start(out=xr8t[:, :, :, sl], in_=xr8_r[:, :, :, sl])
        nc.sync.dma_start(out=wptt, in_=wpt.rearrange("(fc p) o -> p fc o", p=P))

        # ---- persistent q^T, k^T (fp8, 32-feature split), V staging (bf16)
        # and the fp8 V-pair (value + residual k-tiles for DoubleRow PV) ----
        qT8 = qkv.tile([P, 2, T], f8)
        kT8 = qkv.tile([P, 2, T], f8)
        Vp = qkv.tile([P, NTC, F], bf16)
        # Vp2[:, ch, 0] = fp8(V) with a 32.0 ones-column at 64 (Z rides the
        # PV matmul; 32.0 cancels the 32x weight scale); Vp2[:, ch, 1] =
        # fp8 residual with a zero there so Z isn't double-counted. Columns
        # 65..95 pad M to 96 (DoubleRow ldweights needs a multiple of 32).
        Vp2 = qkv.tile([P, NTC, 2, NH_LOC, 96], f8)
        nc.gpsimd.memset(Vp2[:, :, :, :, DH:], 0.0)
        nc.gpsimd.memset(Vp2[:, :, 0, :, DH], 32.0)

        scale = 1.0 / float(DH * 1024)  # muP 1/dh plus the 32*32 fp8 scale
        ps1t = ps_1.tile([P, 2, TQ], f32)
        ps3t = ps_3.tile([P, 2, TQ], f32)

        s1_slot = [0]

        def q_chain(j, half, dst):
            s1_slot[0] ^= 1
            pq = ps1t[:, s1_slot[0], :]
            for kk in range(NKK):
                nc.tensor.matmul(
                    pq,
                    lhsT=wq8t[:, kk, :, half * P:(half + 1) * P] if dst is qT8
                    else wk8t[:, kk, :, half * P:(half + 1) * P],
                    rhs=x8t[:, kk, :, j * TQ:(j + 1) * TQ],
                    start=(kk == 0),
                    stop=(kk == NKK - 1),
                    perf_mode=DR,
                )
            nc.vector.tensor_copy(dst[:, half, j * TQ:(j + 1) * TQ], pq)

        def v_chain(j, r):
            # natural-layout v for key chunk 2j+r via 3-term fp8 residual
            s1_slot[0] ^= 1
            pv = ps1t[:, s1_slot[0], :]
            tsl = slice((2 * j + r) * P, (2 * j + r + 1) * P)
            for term, (xt, wt) in enumerate(
                ((x8t, wv8t), (xr8t, wv8t), (x8t, wvr8t))
            ):
                for kk in range(NKK):
                    nc.tensor.matmul(
                        pv[:, 0:F],
                        lhsT=xt[:, kk, :, tsl],
                        rhs=wt[:, kk, :, :],
                        start=(term == 0 and kk == 0),
                        stop=(term == 2 and kk == NKK - 1),
                        perf_mode=DR,
                    )
            ch = 2 * j + r
            nc.vector.tensor_copy(Vp[:, ch], pv)
            vph = Vp[:, ch].rearrange("p (h c) -> p h c", c=DH)
            nc.gpsimd.tensor_copy(Vp2[:, ch, 0, :, 0:DH], vph)
            nc.gpsimd.tensor_sub(
                Vp2[:, ch, 1, :, 0:DH], vph, Vp2[:, ch, 0, :, 0:DH]
            )

        pending = []

        def proj_q(j, yts, q):
            # row-parallel output projection: one 128-query strip, four
            # [128,256] chunks through the two ps3t slots, paired [128,512]
            # evacuations and early DMAs
            for pair in range(2):
                ob = outp.tile([P, 512], bf16, tag="ob", name=f"ob_{j}_{q}_{pair}")
                for s in range(2):
                    n = 2 * pair + s
                    po = ps3t[:, s, :]
                    for fc in range(2):
                        nc.tensor.matmul(
                            po,
                            lhsT=yts[fc][:, q * P:(q + 1) * P],
                            rhs=wptt[:, fc, n * 256:(n + 1) * 256],
                            start=(fc == 0),
                            stop=(fc == 1),
                        )
                nc.vector.tensor_copy(
                    ob, ps3t.rearrange("p two t -> p (two t)")
                )
                nc.sync.dma_start(
                    out=out[
                        j * TQ + q * P: j * TQ + (q + 1) * P,
                        pair * 512:(pair + 1) * 512,
                    ],
                    in_=ob,
                )

        # prologue: tile 0 needs its projections before its two groups
        q_chain(0, 0, qT8)
        q_chain(0, 1, qT8)
        q_chain(0, 0, kT8)
        q_chain(0, 1, kT8)
        v_chain(0, 0)
        v_chain(0, 1)

        # ---- stage 2: one flat software pipeline over all (j, i) groups.
        # PV(u) is emitted 3 units after S(u) so the in-order PE stream keeps
        # feeding ACT across tile boundaries; v chains, next-tile q/k chains
        # and deferred projection strips fill the remaining TensorE gaps.
        tiles = {}          # j -> (py, yts)

        def y_norm(j):
            # yts = py[0:64] * (1/(32 Z)) partition-broadcast
            py, yts = tiles.pop(j)
            rc = small.tile([1, NH_LOC, TQ], f32, tag="rc")
            nc.vector.reciprocal(rc, py[DH:DH + 1, :, :])
            rb = rbp.tile([DH, NH_LOC, TQ], f32, tag="rb")
            nc.gpsimd.partition_broadcast(rb, rc)
            for h in range(NH_LOC):
                fc, ro = h // 2, (h % 2) * DH
                nc.vector.tensor_mul(
                    yts[fc][ro:ro + DH, :], py[0:DH, h, :], rb[:, h, :]
                )
            for q in range(TQ // P):
                fillers.append((NJ, lambda j=j, yts=yts, q=q: proj_q(j, yts, q)))

        def pv_group(j, i, se, start):
            # fp8 DoubleRow PV: k-tile pair = (V8, Vres8) against the same P
            # tile read twice via a stride-0 dim. Even heads open their bank
            # with start=True, which marks the whole bank pending-zero; the
            # odd head's first write then auto-overwrites (acts as its own
            # start). Group bookkeeping is skipped -- two interleaved chains
            # share a bank.
            py = tiles[j][0]
            w = TQ - start
            for h in range(NH_LOC):
                nc.tensor.matmul(
                    py[:, h, start:],
                    lhsT=Vp2[:, i, :, h, :],
                    rhs=se[:, h, None, start:].broadcast_to([P, 2, w]),
                    start=(i == 0 and h % 2 == 0),
                    stop=False,
                    skip_group_check=True,
                    perf_mode=DR,
                )
            if i == 2 * j + 1:
                y_norm(j)

        fillers = []        # (deadline_tile, closure), popped one per unit
        pv_q = []
        units = [(j, i) for j in range(NJ) for i in range(2 * j + 2)]
        for (j, i) in units:
            if i == 0:
                tiles[j] = (
                    ps_y.tile([96, NH_LOC, TQ], f32, tag="y",
                              name=f"py_{j}"),
                    [ytp.tile([P, TQ], bf16, tag=f"yt{fc}", name=f"yt{fc}_{j}")
                     for fc in range(2)],
                )
                if j > 0:
                    # V chunks 2j, 2j+1 are read by this tile's last two
                    # groups; the q/k chains feed tile j+1's first group
                    fillers.append((j, lambda j=j: v_chain(j, 0)))
                    fillers.append((j, lambda j=j: v_chain(j, 1)))
                if j + 1 < NJ:
                    for half, dst in ((0, qT8), (1, qT8), (0, kT8), (1, kT8)):
                        fillers.append(
                            (j + 1,
                             lambda j=j, half=half, dst=dst:
                                 q_chain(j + 1, half, dst))
                        )
                # flush anything that must precede this tile's groups
                overdue = [f for (dl, f) in fillers if dl <= j]
                fillers = [(dl, f) for (dl, f) in fillers if dl > j]
                for f in overdue:
                    f()

            d = i - 2 * j
            start = 0 if d <= 0 else 128
            ps = ps_s.tile([P, NH_LOC, TQ], f32, tag="s")
            for h in range(NH_LOC):
                nc.tensor.matmul(
                    ps[:, h, start:],
                    lhsT=kT8[32 * h:32 * h + 32, :, i * P:(i + 1) * P],
                    rhs=qT8[32 * h:32 * h + 32, :, j * TQ + start:(j + 1) * TQ],
                    start=True,
                    stop=True,
                    perf_mode=DR,
                    tile_position=(32 * h, 0),
                )
            se = sexp.tile([P, NH_LOC, TQ], f8, tag="se")
            nc.scalar.activation(se[:, :, start:], ps[:, :, start:], EXP,
                                 scale=scale)
            if d == 0:
                nc.gpsimd.tensor_mul(se[:, :, 0:P], se[:, :, 0:P], m4)
            elif d == 1:
                nc.gpsimd.tensor_mul(se[:, :, P:TQ], se[:, :, P:TQ], m4)
            pv_q.append((j, i, se, start))
            if len(pv_q) > 3:
                pv_group(*pv_q.pop(0))
            if fillers:
                fillers.pop(0)[1]()

        for args in pv_q:
            pv_group(*args)
        for _, fill in fillers:
            fill()


_CACHE = {}


def _perm():
    # feature permutation for the 32-feature-split q/k layout:
    # column (half*128 + 32h + f) holds feature (64h + 32*half + f)
    perm = np.zeros(F, dtype=np.int64)
    for half in range(2):
        for h in range(NH_LOC):
            for f in range(32):
                perm[half * 128 + 32 * h + f] = 64 * h + 32 * half + f
    return perm


def shard_inputs(x, wq, wk, wv, wproj):
    perm = _perm()
    in_maps = []
    for c in range(N_CORES):
        b, g = divmod(c, GROUPS)
        sl = slice(g * F, (g + 1) * F)
        xb = np.ascontiguousarray(x[b].T, dtype=np.float32)  # [C, T]
        x8 = xb.astype(F8)
        xr8 = (xb - x8.astype(np.float32)).astype(F8)
        wqt = 32.0 * wq[sl, :].T  # [C, F]
        wkt = 32.0 * wk[sl, :].T
        wvt = 32.0 * wv[sl, :].T
        wv8 = wvt.astype(F8)
        wvr8 = (wvt - wv8.astype(np.float32)).astype(F8)
        in_maps.append(
            {
                "x8": x8,
                "xr8": xr8,
                "wq8": np.ascontiguousarray(wqt[:, perm]).astype(F8),
                "wk8": np.ascontiguousarray(wkt[:, perm]).astype(F8),
                "wv8": wv8,
                "wvr8": wvr8,
                "wpt": np.ascontiguousarray(wproj[:, sl].T).astype(BF),
            }
        )
    return in_maps


def kernel(x, wq, wk, wv, wproj):
    x = np.asarray(x, dtype=np.float32)
    wq = np.asarray(wq, dtype=np.float32)
    wk = np.asarray(wk, dtype=np.float32)
    wv = np.asarray(wv, dtype=np.float32)
    wproj = np.asarray(wproj, dtype=np.float32)

    from concourse._compat import axon_active

    if axon_active():
        # the axon NTFF-profile hook isn't available in this environment;
        # a BASS_TRACE=1 run would crash importing it, so disable tracing
        os.environ.setdefault("BASS_NEVER_TRACE", "1")

    if "nc" not in _CACHE:
        _CACHE["nc"] = build_module()
    nc = _CACHE["nc"]

    in_maps = shard_inputs(x, wq, wk, wv, wproj)
    res = run_bass_kernel_spmd(nc, in_maps, core_ids=list(range(N_CORES)))
    out = np.zeros((B, T, C), np.float32)
    for c in range(N_CORES):
        b = c // GROUPS
        out[b] += res.results[c]["out"].astype(np.float32)
    return out


# revision 19
# speedup vs baseline: 1.0713x; 1.0278x over previous
"""Causal self-attention (muP scaling) for Trainium2, sharded over 8 NeuronCores.

Sharding: data-parallel over batch (B=2) x tensor-parallel over head groups
(16 heads -> 4 groups of 4). Core c handles batch c//4, head group c%4.
Each core computes q/k/v projections for its 256 features, causal attention
for its 4 heads, and a row-parallel partial of the output projection
(written back in bf16; the host sums the 4 partials per batch element).

Dtype strategy (validated against the 2e-2 tolerance; measured 5.8e-3):
 - q,k projections and S=K^T Q run in fp8e4 with the DoubleRow perf mode
   (two 128-deep k-tiles contracted per pass). Weights are pre-scaled by
   32 on the host so their values sit in fp8's sweet spot; the 32*32
   factor is folded into the exp scale. S contracts the 64 features of a
   head as two 32-partition k-tiles, using PE tiling at base partition
   32h, which requires the q/k features laid out [32h + f%32, f//32] --
   the host permutes wq/wk columns accordingly.
 - v projection runs in fp8 DoubleRow with residual-stacked operand pairs
   (x8*wv8 + xr8*wv8 + x8*wvr8), giving bf16-grade accuracy at 0.75x the
   bf16 cost. The 32x weight scale cancels out through the softmax
   normalisation because the ones-column in V' is set to 32.0.
 - exp output (P), V', y and wproj are bf16; PV and the output projection
   are bf16 matmuls. PSUM accumulation is f32 throughout.

Pipeline per 256-query tile j (Tq=256 so S PSUM groups are 2 banks and can
be double-buffered): q/k/v chains -> per key-chunk i: 4 S matmuls into a
[128,4,256] PSUM group, one batched exp over all 4 heads straight out of
PSUM (bf16 out), causal masking only on the two diagonal chunks, 4 PV
accumulations into a [65,4,256] PSUM group whose ones-row yields the
softmax denominator for free. Deferred output-projection chunks of tile
j-1 are interleaved between key-chunks to fill TensorE gaps while ACT
grinds exp.
"""

import os
import sys

for _p in ("/opt/trn_rl_repo",):
    if _p not in sys.path:
        sys.path.insert(0, _p)

import numpy as np
import ml_dtypes

import concourse.bass as bass  # noqa: F401
import concourse.mybir as mybir
import concourse.tile as tile
from concourse import bacc
from concourse.bass_utils import run_bass_kernel_spmd
from concourse.masks import make_upper_triangular
from concourse.tile import ScopedClock

# ---- problem constants (hardcoded per contract) ----
B, T, C = 2, 2048, 1024
NH, DH = 16, 64
N_CORES = 8
GROUPS = 4                 # head groups (tensor parallel)
NH_LOC = NH // GROUPS      # 4 heads per core
F = NH_LOC * DH            # 256 per-core qkv features
P = 128
TQ = 256                   # query tile
NJ = T // TQ               # 8 query tiles
NTC = T // P               # 16 key chunks of 128
NKK = C // 256             # 4 DoubleRow k-tile pairs over C
f32 = mybir.dt.float32
bf16 = mybir.dt.bfloat16
f8 = mybir.dt.float8e4
EXP = mybir.ActivationFunctionType.Exp
DR = mybir.MatmulPerfMode.DoubleRow
F8 = ml_dtypes.float8_e4m3
BF = ml_dtypes.bfloat16


def _install_drain_patch():
    """This walrus build rejects >2 sem waits on a single instruction; the
    Tile tail drain accumulates one wait per live proc. Split them into
    single-wait SP nops ahead of the drain."""
    if getattr(tile.TileContext, "_drain_patch_installed", False):
        return

    def _patched(self, tick_clock, wait_clock):
        nc = self.nc
        probe = nc.sync.nop(nofuse=True)
        wait_clock.add_sem_waits(
            probe.ins, ScopedClock({None: tick_clock.global_clock})
        )
        si = probe.ins.sync_info
        waits = list(si.on_wait) if si is not None and si.on_wait else []
        if len(waits) > 1:
            probe.ins.sync_info.on_wait = [waits[0]]
            for w in waits[1:]:
                n2 = nc.sync.nop(nofuse=True)
                n2.ins.sync_info = mybir.SyncInfo(on_wait=[w], on_update=[])
        nc.sync.drain()
        nc.all_engine_barrier()
        assert self.sems is not None
        popped = nc._tile_sem_poison_stack.pop()
        assert popped is self._sem_poison
        nc.clear_and_free_semaphores(list(self.sems.allocated().values()))
        nc.all_engine_barrier()

    tile.TileContext._drain_and_barrier = _patched
    tile.TileContext._drain_patch_installed = True


def build_module():
    """Build the per-core Bass module (uniform across all 8 cores)."""
    _install_drain_patch()
    nc = bacc.Bacc("TRN2", target_bir_lowering=False, debug=False)
    x8 = nc.dram_tensor("x8", [C, T], f8, kind="ExternalInput").ap()
    xr8 = nc.dram_tensor("xr8", [C, T], f8, kind="ExternalInput").ap()
    wq8 = nc.dram_tensor("wq8", [C, F], f8, kind="ExternalInput").ap()
    wk8 = nc.dram_tensor("wk8", [C, F], f8, kind="ExternalInput").ap()
    wv8 = nc.dram_tensor("wv8", [C, F], f8, kind="ExternalInput").ap()
    wvr8 = nc.dram_tensor("wvr8", [C, F], f8, kind="ExternalInput").ap()
    wpt = nc.dram_tensor("wpt", [F, C], bf16, kind="ExternalInput").ap()
    out = nc.dram_tensor("out", [T, C], bf16, kind="ExternalOutput").ap()

    with tile.TileContext(nc) as tc:
        _body(tc, x8, xr8, wq8, wk8, wv8, wvr8, wpt, out)
    nc.compile()
    return nc


def _body(tc, x8, xr8, wq8, wk8, wv8, wvr8, wpt, out):
    from contextlib import ExitStack

    nc = tc.nc
    with ExitStack() as ctx:
        const = ctx.enter_context(tc.tile_pool(name="const", bufs=1))
        wpool = ctx.enter_context(tc.tile_pool(name="wpool", bufs=1))
        qkv = ctx.enter_context(tc.tile_pool(name="qkv", bufs=1))
        sexp = ctx.enter_context(tc.tile_pool(name="sexp", bufs=8))
        ytp = ctx.enter_context(tc.tile_pool(name="ytp", bufs=4))
        rbp = ctx.enter_context(tc.tile_pool(name="rbp", bufs=3))
        small = ctx.enter_context(tc.tile_pool(name="small", bufs=3))
        outp = ctx.enter_context(tc.tile_pool(name="outp", bufs=6))
        ps_s = ctx.enter_context(tc.tile_pool(name="ps_s", bufs=2, space="PSUM"))
        ps_y = ctx.enter_context(tc.tile_pool(name="ps_y", bufs=1, space="PSUM"))
        ps_1 = ctx.enter_context(tc.tile_pool(name="ps_1", bufs=1, space="PSUM"))
        ps_3 = ctx.enter_context(tc.tile_pool(name="ps_3", bufs=1, space="PSUM"))

        # causal mask for the diagonal 128-blocks, replicated over 4 heads
        umask = const.tile([P, P], f8)
        make_upper_triangular(nc, umask, val=1.0, diag=True)
        m4 = const.tile([P, NH_LOC, P], f8)
        for h in range(NH_LOC):
            nc.gpsimd.tensor_copy(m4[:, h, :], umask)

        # ---- weights + x straight from HBM (pre-transposed, fp8/bf16) ----
        wq8t = wpool.tile([P, NKK, 2, F], f8)
        wk8t = wpool.tile([P, NKK, 2, F], f8)
        wv8t = wpool.tile([P, NKK, 2, F], f8)
        wvr8t = wpool.tile([P, NKK, 2, F], f8)
        wptt = wpool.tile([P, 2, C], bf16)
        x8t = wpool.tile([P, NKK, 2, T], f8)
        xr8t = wpool.tile([P, NKK, 2, T], f8)

        wq_r = wq8.rearrange("(kk two p) m -> p kk two m", p=P, two=2)
        wk_r = wk8.rearrange("(kk two p) m -> p kk two m", p=P, two=2)
        wv_r = wv8.rearrange("(kk two p) m -> p kk two m", p=P, two=2)
        wvr_r = wvr8.rearrange("(kk two p) m -> p kk two m", p=P, two=2)
        x8_r = x8.rearrange("(kk two p) t -> p kk two t", p=P, two=2)
        xr8_r = xr8.rearrange("(kk two p) t -> p kk two t", p=P, two=2)

        sl = slice(0, 256)
        nc.sync.dma_start(out=wq8t[:, :, :, 0:P], in_=wq_r[:, :, :, 0:P])
        nc.sync.dma_start(out=x8t[:, :, :, sl], in_=x8_r[:, :, :, sl])
        nc.sync.dma_start(out=wk8t[:, :, :, 0:P], in_=wk_r[:, :, :, 0:P])
        nc.sync.dma_start(out=wq8t[:, :, :, P:F], in_=wq_r[:, :, :, P:F])
        nc.sync.dma_start(out=wk8t[:, :, :, P:F], in_=wk_r[:, :, :, P:F])
        nc.sync.dma_start(out=xr8t[:, :, :, sl], in_=xr8_r[:, :, :, sl])
        nc.sync.dma_start(out=wv8t, in_=wv_r)
        nc.sync.dma_start(out=wvr8t, in_=wvr_r)
        for lo, hi in ((256, 512), (512, 1024), (1024, 2048)):
            sl = slice(lo, hi)
            nc.sync.dma_start(out=x8t[:, :, :, sl], in_=x8_r[:, :, :, sl])
            nc.sync.dma_start(out=xr8t[:, :, :, sl], in_=xr8_r[:, :, :, sl])
        nc.sync.dma_start(out=wptt, in_=wpt.rearrange("(fc p) o -> p fc o", p=P))

        # ---- persistent q^T, k^T (fp8, 32-feature split), V staging (bf16)
        # and the fp8 V-pair (value + residual k-tiles for DoubleRow PV) ----
        qT8 = qkv.tile([P, 2, T], f8)
        kT8 = qkv.tile([P, 2, T], f8)
        Vp = qkv.tile([P, NTC, F], bf16)
        # Vp2[:, ch, 0] = fp8(V) with a 32.0 ones-column at 64 (Z rides the
        # PV matmul; 32.0 cancels the 32x weight scale); Vp2[:, ch, 1] =
        # fp8 residual with a zero there so Z isn't double-counted. Columns
        # 65..95 pad M to 96 (DoubleRow ldweights needs a multiple of 32).
        Vp2 = qkv.tile([P, NTC, 2, NH_LOC, 96], f8)
        nc.gpsimd.memset(Vp2[:, :, :, :, DH:], 0.0)
        nc.gpsimd.memset(Vp2[:, :, 0, :, DH], 32.0)

        scale = 1.0 / float(DH * 1024)  # muP 1/dh plus the 32*32 fp8 scale
        ps1t = ps_1.tile([P, 2, TQ], f32)
        ps3t = ps_3.tile([P, 2, TQ], f32)

        s1_slot = [0]

        def q_chain(j, half, dst, pq=None):
            if pq is None:
                s1_slot[0] ^= 1
                pq = ps1t[:, s1_slot[0], :]
            for kk in range(NKK):
                nc.tensor.matmul(
                    pq,
                    lhsT=wq8t[:, kk, :, half * P:(half + 1) * P] if dst is qT8
                    else wk8t[:, kk, :, half * P:(half + 1) * P],
                    rhs=x8t[:, kk, :, j * TQ:(j + 1) * TQ],
                    start=(kk == 0),
                    stop=(kk == NKK - 1),
                    perf_mode=DR,
                )
            nc.vector.tensor_copy(dst[:, half, j * TQ:(j + 1) * TQ], pq)

        def v_chain(j, r):
            # natural-layout v for key chunk 2j+r via 3-term fp8 residual
            s1_slot[0] ^= 1
            pv = ps1t[:, s1_slot[0], :]
            tsl = slice((2 * j + r) * P, (2 * j + r + 1) * P)
            for term, (xt, wt) in enumerate(
                ((x8t, wv8t), (xr8t, wv8t), (x8t, wvr8t))
            ):
                for kk in range(NKK):
                    nc.tensor.matmul(
                        pv[:, 0:F],
                        lhsT=xt[:, kk, :, tsl],
                        rhs=wt[:, kk, :, :],
                        start=(term == 0 and kk == 0),
                        stop=(term == 2 and kk == NKK - 1),
                        perf_mode=DR,
                    )
            ch = 2 * j + r
            nc.vector.tensor_copy(Vp[:, ch], pv)
            vph = Vp[:, ch].rearrange("p (h c) -> p h c", c=DH)
            nc.gpsimd.tensor_copy(Vp2[:, ch, 0, :, 0:DH], vph)
            nc.gpsimd.tensor_sub(
                Vp2[:, ch, 1, :, 0:DH], vph, Vp2[:, ch, 0, :, 0:DH]
            )

        pending = []

        def proj_q(j, yts, q, tail=False):
            # row-parallel output projection: one 128-query strip, four
            # [128,256] chunks, paired [128,512] evacuations and early DMAs.
            # In tail mode (after the last attention group) the strip runs
            # through a freed 4-slot attention-PSUM tile so the chains are
            # not serialised by evacuation round-trips.
            pstile = ps_s.tile([P, NH_LOC, TQ], f32, tag="s",
                               name=f"pot_{j}_{q}") if tail else None
            for pair in range(2):
                ob = outp.tile([P, 512], bf16, tag="ob", name=f"ob_{j}_{q}_{pair}")
                for s in range(2):
                    n = 2 * pair + s
                    po = pstile[:, n, :] if tail else ps3t[:, s, :]
                    for fc in range(2):
                        nc.tensor.matmul(
                            po,
                            lhsT=yts[fc][:, q * P:(q + 1) * P],
                            rhs=wptt[:, fc, n * 256:(n + 1) * 256],
                            start=(fc == 0),
                            stop=(fc == 1),
                        )
                src_pair = (pstile[:, 2 * pair:2 * pair + 2, :] if tail
                            else ps3t)
                nc.vector.tensor_copy(
                    ob, src_pair.rearrange("p two t -> p (two t)")
                )
                nc.sync.dma_start(
                    out=out[
                        j * TQ + q * P: j * TQ + (q + 1) * P,
                        pair * 512:(pair + 1) * 512,
                    ],
                    in_=ob,
                )

        # prologue: tile 0's q/k chains on four distinct PSUM slots (no
        # evacuation round-trip between them; proj hasn't started yet), with
        # v chains as fillers so the first S groups aren't blocked on xr8
        q_chain(0, 0, qT8, pq=ps1t[:, 0, :])
        q_chain(0, 1, qT8, pq=ps1t[:, 1, :])
        q_chain(0, 0, kT8, pq=ps3t[:, 0, :])
        q_chain(0, 1, kT8, pq=ps3t[:, 1, :])

        # ---- stage 2: one flat software pipeline over all (j, i) groups.
        # PV(u) is emitted 3 units after S(u) so the in-order PE stream keeps
        # feeding ACT across tile boundaries; v chains, next-tile q/k chains
        # and deferred projection strips fill the remaining TensorE gaps.
        tiles = {}          # j -> (py, yts)

        def y_norm(j):
            # yts = py[0:64] * (1/(32 Z)) partition-broadcast
            py, yts = tiles.pop(j)
            rc = small.tile([1, NH_LOC, TQ], f32, tag="rc")
            nc.vector.reciprocal(rc, py[DH:DH + 1, :, :])
            rb = rbp.tile([DH, NH_LOC, TQ], f32, tag="rb")
            nc.gpsimd.partition_broadcast(rb, rc)
            for h in range(NH_LOC):
                fc, ro = h // 2, (h % 2) * DH
                nc.vector.tensor_mul(
                    yts[fc][ro:ro + DH, :], py[0:DH, h, :], rb[:, h, :]
                )
            tail = (j == NJ - 1)
            for q in range(TQ // P):
                fillers.append(
                    (NJ, lambda j=j, yts=yts, q=q, tail=tail:
                        proj_q(j, yts, q, tail=tail))
                )

        def pv_group(j, i, se, start):
            # fp8 DoubleRow PV: k-tile pair = (V8, Vres8) against the same P
            # tile read twice via a stride-0 dim. Even heads open their bank
            # with start=True, which marks the whole bank pending-zero; the
            # odd head's first write then auto-overwrites (acts as its own
            # start). Group bookkeeping is skipped -- two interleaved chains
            # share a bank.
            py = tiles[j][0]
            w = TQ - start
            for h in range(NH_LOC):
                nc.tensor.matmul(
                    py[:, h, start:],
                    lhsT=Vp2[:, i, :, h, :],
                    rhs=se[:, h, None, start:].broadcast_to([P, 2, w]),
                    start=(i == 0 and h % 2 == 0),
                    stop=False,
                    skip_group_check=True,
                    perf_mode=DR,
                )
            if i == 2 * j + 1:
                y_norm(j)

        fillers = []        # (deadline_tile, closure), popped one per unit
        pv_q = []
        units = [(j, i) for j in range(NJ) for i in range(2 * j + 2)]
        for (j, i) in units:
            if i == 0:
                tiles[j] = (
                    ps_y.tile([96, NH_LOC, TQ], f32, tag="y",
                              name=f"py_{j}"),
                    [ytp.tile([P, TQ], bf16, tag=f"yt{fc}", name=f"yt{fc}_{j}")
                     for fc in range(2)],
                )
                # V chunks 2j, 2j+1 are read by this tile's last two
                # groups (lagged 3 units, so deadline j+1 suffices); the
                # q/k chains feed tile j+1's first group
                fillers.append((j + 1, lambda j=j: v_chain(j, 0)))
                fillers.append((j + 1, lambda j=j: v_chain(j, 1)))
                if j + 1 < NJ:
                    for half, dst in ((0, qT8), (1, qT8), (0, kT8), (1, kT8)):
                        fillers.append(
                            (j + 1,
                             lambda j=j, half=half, dst=dst:
                                 q_chain(j + 1, half, dst))
                        )
                # flush anything that must precede this tile's groups
                overdue = [f for (dl, f) in fillers if dl <= j]
                fillers = [(dl, f) for (dl, f) in fillers if dl > j]
                for f in overdue:
                    f()

            d = i - 2 * j
            start = 0 if d <= 0 else 128
            ps = ps_s.tile([P, NH_LOC, TQ], f32, tag="s")
            for h in range(NH_LOC):
                nc.tensor.matmul(
                    ps[:, h, start:],
                    lhsT=kT8[32 * h:32 * h + 32, :, i * P:(i + 1) * P],
                    rhs=qT8[32 * h:32 * h + 32, :, j * TQ + start:(j + 1) * TQ],
                    start=True,
                    stop=True,
                    perf_mode=DR,
                    tile_position=(32 * h, 0),
                )
            se = sexp.tile([P, NH_LOC, TQ], f8, tag="se")
            nc.scalar.activation(se[:, :, start:], ps[:, :, start:], EXP,
                                 scale=scale)
            eng = nc.vector if j == NJ - 1 else nc.gpsimd
            if d == 0:
                eng.tensor_mul(se[:, :, 0:P], se[:, :, 0:P], m4)
            elif d == 1:
                eng.tensor_mul(se[:, :, P:TQ], se[:, :, P:TQ], m4)
            pv_q.append((j, i, se, start))
            if len(pv_q) > 3:
                pv_group(*pv_q.pop(0))
            if fillers:
                fillers.pop(0)[1]()

        for args in pv_q:
            pv_group(*args)
        for _, fill in fillers:
            fill()


_CACHE = {}


def _perm():
    # feature permutation for the 32-feature-split q/k layout:
    # column (half*128 + 32h + f) holds feature (64h + 32*half + f)
    perm = np.zeros(F, dtype=np.int64)
    for half in range(2):
        for h in range(NH_LOC):
            for f in range(32):
                perm[half * 128 + 32 * h + f] = 64 * h + 32 * half + f
    return perm


def shard_inputs(x, wq, wk, wv, wproj):
    perm = _perm()
    in_maps = []
    for c in range(N_CORES):
        b, g = divmod(c, GROUPS)
        sl = slice(g * F, (g + 1) * F)
        xb = np.ascontiguousarray(x[b].T, dtype=np.float32)  # [C, T]
        x8 = xb.astype(F8)
        xr8 = (xb - x8.astype(np.float32)).astype(F8)
        wqt = 32.0 * wq[sl, :].T  # [C, F]
        wkt = 32.0 * wk[sl, :].T
        wvt = 32.0 * wv[sl, :].T
        wv8 = wvt.astype(F8)
        wvr8 = (wvt - wv8.astype(np.float32)).astype(F8)
        in_maps.append(
            {
                "x8": x8,
                "xr8": xr8,
                "wq8": np.ascontiguousarray(wqt[:, perm]).astype(F8),
                "wk8": np.ascontiguousarray(wkt[:, perm]).astype(F8),
                "wv8": wv8,
                "wvr8": wvr8,
                "wpt": np.ascontiguousarray(wproj[:, sl].T).astype(BF),
            }
        )
    return in_maps


def kernel(x, wq, wk, wv, wproj):
    x = np.asarray(x, dtype=np.float32)
    wq = np.asarray(wq, dtype=np.float32)
    wk = np.asarray(wk, dtype=np.float32)
    wv = np.asarray(wv, dtype=np.float32)
    wproj = np.asarray(wproj, dtype=np.float32)

    from concourse._compat import axon_active

    if axon_active():
        # the axon NTFF-profile hook isn't available in this environment;
        # a BASS_TRACE=1 run would crash importing it, so disable tracing
        os.environ.setdefault("BASS_NEVER_TRACE", "1")

    if "nc" not in _CACHE:
        _CACHE["nc"] = build_module()
    nc = _CACHE["nc"]

    in_maps = shard_inputs(x, wq, wk, wv, wproj)
    res = run_bass_kernel_spmd(nc, in_maps, core_ids=list(range(N_CORES)))
    out = np.zeros((B, T, C), np.float32)
    for c in range(N_CORES):
        b = c // GROUPS
        out[b] += res.results[c]["out"].astype(np.float32)
    return out
